# revision 32
# baseline (speedup 1.0000x reference)
"""Additive-attention pooling kernel for Trainium2 (8 NeuronCores, SPMD).

Reference math (per example b):
    kp     = k[b] @ Wk.T + bk          [S, A]
    qp     = q[b] @ Wq.T + bq          [A]
    hidden = tanh(kp + qp)             [S, A]
    energy = hidden @ Wa (+ ba)        [S]     (ba is a constant shift ->
                                                softmax-invariant, dropped)
    score  = softmax(energy)           [S]
    attn   = score @ v[b]              [1, V]

Sharding: data-parallel over batch B=64 -> 8 examples per core, weights
replicated, no cross-core communication.

Per-core dataflow (E=8 examples):
  - k, v streamed from HBM with inline f32->bf16 cast (SWDGE dma).
  - k tiles PE-transposed (contraction dim must sit on partitions).
  - kpT[a, s] = sum_kc WkT[kc,a].T @ kT[kc,s] accumulated in PSUM.
  - hiddenT = tanh(kpT + biasT[:,e]) on ScalarE (per-partition bias holds
    bk + bq + (q @ Wq.T) for this example).
  - energy[1, s] = Wa.T @ hiddenT accumulated over a-chunks.
  - per-pair softmax on the free dim, scores PE-transposed, then
    attn[1, V] = sum_sc scoreT[:,e].T @ v_tile accumulated in PSUM.
"""

import sys

sys.path.insert(0, "/opt/trn_rl_repo")

import numpy as np

# Problem geometry (hardcoded per spec).
B = 64          # global batch
NCORES = 8
E = B // NCORES  # examples per core = 8
S = 2048        # sequence length
D = 512         # k_dim = q_dim = a_dim = v_dim
P = 128         # partitions
NKC = D // P    # contraction chunks = 4
NAC = D // P    # a chunks = 4
SCW = 512       # s-chunk width for the main matmul
NSC = S // SCW  # s-chunks per example = 4
NPAIR = E // 2  # softmax/attn processed per example-pair

_CACHE = {}


def _build(reps=1):
    """Build + compile the per-core Bass program (cached per process).

    reps>1 unrolls the whole main pipeline N times inside the NEFF (same
    inputs/outputs) — used by test.py to measure kernel time as the slope
    (T_N - T_1)/(N-1), cancelling the large per-dispatch axon overhead.

    """
    import concourse.bass as bass  # noqa: F401
    import concourse.tile as tile
    from concourse import bacc, mybir
    from concourse.masks import make_identity
    from contextlib import ExitStack

    f32 = mybir.dt.float32
    bf16 = mybir.dt.bfloat16
    AF = mybir.ActivationFunctionType
    AX = mybir.AxisListType

    nc = bacc.Bacc(
        "TRN2",
        target_bir_lowering=False,
        debug=False,
        enable_asserts=True,
        num_devices=NCORES,
    )

    k_d = nc.dram_tensor("k", [E, S, D], f32, kind="ExternalInput").ap()
    v_d = nc.dram_tensor("v", [E, S, D], f32, kind="ExternalInput").ap()
    q_d = nc.dram_tensor("q", [E, D], f32, kind="ExternalInput").ap()
    wk_d = nc.dram_tensor("Wk", [D, D], f32, kind="ExternalInput").ap()
    bk_d = nc.dram_tensor("bk", [D], f32, kind="ExternalInput").ap()
    wq_d = nc.dram_tensor("Wq", [D, D], f32, kind="ExternalInput").ap()
    bq_d = nc.dram_tensor("bq", [D], f32, kind="ExternalInput").ap()
    wa_d = nc.dram_tensor("Wa", [D], f32, kind="ExternalInput").ap()

    score_o = nc.dram_tensor("score", [E, S], f32, kind="ExternalOutput").ap()
    attn_o = nc.dram_tensor("attn", [E, 1, D], f32, kind="ExternalOutput").ap()

    with tile.TileContext(nc) as tc, ExitStack() as ctx:
        const = ctx.enter_context(tc.tile_pool(name="const", bufs=1))

        ident = const.tile([P, P], bf16)
        make_identity(nc, ident[:])
        wkT = const.tile([P, NKC, D], bf16)  # [kk_p, kc, a]
        biasT = const.tile([P, NAC, E], f32)
        wa_sb = const.tile([P, NAC], bf16)

        # ---- main pipeline pools (created first: stack-allocated below
        # the transient setup pools) ------------------------------------
        kpool = ctx.enter_context(tc.tile_pool(name="kpool", bufs=4))
        ktpool = ctx.enter_context(tc.tile_pool(name="ktpool", bufs=4))
        vpool = ctx.enter_context(tc.tile_pool(name="vpool", bufs=20))
        hpool = ctx.enter_context(tc.tile_pool(name="hpool", bufs=20))
        epool = ctx.enter_context(tc.tile_pool(name="epool", bufs=2))
        smpool = ctx.enter_context(tc.tile_pool(name="smpool", bufs=1))

        def load_kv(e, sc):
            """Issue the (cast) k/v chunk loads for (e, sc): one 8KB run
            per partition (s-local = 4*p + t)."""
            k_t = kpool.tile([P, NSC, D], bf16, tag="k")
            nc.gpsimd.dma_start(
                k_t[:],
                k_d[e, sc * SCW : (sc + 1) * SCW, :].rearrange(
                    "(p t) kk -> p t kk", p=P
                ),
            )
            v_t = vpool.tile([P, NSC, D], bf16, tag="v")
            nc.gpsimd.dma_start(
                v_t[:],
                v_d[e, sc * SCW : (sc + 1) * SCW, :].rearrange(
                    "(p t) vv -> p t vv", p=P
                ),
            )
            return k_t, v_t

        # prefetch example 0 ahead of the weight preprocessing
        prefetched = {}
        for sc in range(NSC):
            prefetched[sc] = load_kv(0, sc)

        # ---- weight preprocessing (scoped pools: freed before main) ----
        with ExitStack() as sctx:
            setup = sctx.enter_context(tc.tile_pool(name="setup", bufs=2))
            ps_setup = sctx.enter_context(
                tc.tile_pool(name="ps_setup", bufs=2, space="PSUM")
            )

            # WkT[kk, a]: PE-transpose of bf16 Wk blocks.
            wk_nat = setup.tile([P, NAC, D], bf16, tag="w_nat")
            nc.gpsimd.dma_start(
                wk_nat[:], wk_d.rearrange("(ab p) kk -> p ab kk", p=P)
            )
            for ab in range(NAC):
                for kc in range(NKC):
                    ps = ps_setup.tile([P, P], bf16, tag="s_ps")
                    nc.tensor.transpose(
                        ps[:], wk_nat[:, ab, kc * P : (kc + 1) * P], ident[:]
                    )
                    nc.vector.tensor_copy(wkT[:, kc, ab * P : (ab + 1) * P], ps[:])

            wq_nat = setup.tile([P, NAC, D], bf16, tag="w_nat")
            nc.gpsimd.dma_start(
                wq_nat[:], wq_d.rearrange("(ab p) qq -> p ab qq", p=P)
            )
            wqT = setup.tile([P, NKC, D], bf16, tag="wqT")  # [qq_p, qc, a]
            for ab in range(NAC):
                for qc in range(NKC):
                    ps = ps_setup.tile([P, P], bf16, tag="s_ps")
                    nc.tensor.transpose(
                        ps[:], wq_nat[:, ab, qc * P : (qc + 1) * P], ident[:]
                    )
                    nc.vector.tensor_copy(wqT[:, qc, ab * P : (ab + 1) * P], ps[:])

            # qT[qq, e]
            q_sb = setup.tile([E, D], bf16, tag="q_sb")
            nc.gpsimd.dma_start(q_sb[:], q_d)
            qT = setup.tile([P, NKC, E], bf16, tag="qT")
            for qc in range(NKC):
                ps = ps_setup.tile([P, E], bf16, tag="s_ps")
                nc.tensor.transpose(
                    ps[:], q_sb[:, qc * P : (qc + 1) * P], ident[:E, :E]
                )
                nc.vector.tensor_copy(qT[:, qc, :], ps[:])

            # biasvec[a] = bk[a] + bq[a], laid out [p, ac] (HWDGE: no cast)
            bk_sb = setup.tile([P, NAC], f32, tag="bk")
            nc.sync.dma_start(bk_sb[:], bk_d.rearrange("(ac p) -> p ac", p=P))
            bq_sb = setup.tile([P, NAC], f32, tag="bq")
            nc.sync.dma_start(bq_sb[:], bq_d.rearrange("(ac p) -> p ac", p=P))
            biasvec = setup.tile([P, NAC], f32, tag="biasvec")
            nc.vector.tensor_add(biasvec[:], bk_sb[:], bq_sb[:])

            # biasT[a, e] = (q @ Wq.T)[e, a] + bk[a] + bq[a]
            for ac in range(NAC):
                pq = ps_setup.tile([P, E], f32, tag="s_ps")
                for qc in range(NKC):
                    nc.tensor.matmul(
                        pq[:],
                        wqT[:, qc, ac * P : (ac + 1) * P],
                        qT[:, qc, :],
                        start=(qc == 0),
                        stop=(qc == NKC - 1),
                    )
                nc.vector.tensor_scalar_add(
                    biasT[:, ac, :], pq[:], biasvec[:, ac : ac + 1]
                )

            # Wa as lhsT chunks [a_p, ac] (bf16 for the energy matmul)
            nc.gpsimd.dma_start(wa_sb[:], wa_d.rearrange("(ac p) -> p ac", p=P))

        # ---- main pipeline --------------------------------------------
        tr_ps = ctx.enter_context(tc.tile_pool(name="tr_ps", bufs=3, space="PSUM"))
        mm_ps = ctx.enter_context(tc.tile_pool(name="mm_ps", bufs=3, space="PSUM"))
        en_ps = ctx.enter_context(tc.tile_pool(name="en_ps", bufs=2, space="PSUM"))

        v_tiles = {}

        NQ = 4  # examples per tail group; energy rows at partitions 0/32/64/96
        NPG = 32 * (NQ - 1) + 1  # 97 partitions, rows at 0/32/64/96
        for rep in range(reps):
          for gr in range(E // NQ):
            e0 = NQ * gr
            energies = epool.tile([NPG, S], f32, tag="energies")
            nc.vector.memset(energies[:], 0.0)

            for ei in range(NQ):
                e = e0 + ei
                hid_tiles = {}
                for sc in range(NSC):
                    if rep == 0 and e == 0:
                        k_t, v_t = prefetched[sc]
                    else:
                        k_t, v_t = load_kv(e, sc)
                    v_tiles[(e, sc)] = v_t

                    # transpose k: kT[kk, kc, s]
                    kT = ktpool.tile([P, NKC, SCW], bf16, tag="kT")
                    for t in range(NSC):
                        trp = tr_ps.tile([P, NKC, P], bf16, tag="tr")
                        for kc in range(NKC):
                            nc.tensor.transpose(
                                trp[:, kc, :],
                                k_t[:, t, kc * P : (kc + 1) * P],
                                ident[:],
                            )
                        nc.vector.tensor_copy(kT[:, :, t * P : (t + 1) * P], trp[:])

                    # kpT[a, s] + tanh (hid kept for the per-example energy)
                    for ac in range(NAC):
                        kp_ps = mm_ps.tile([P, SCW], f32, tag="kp")
                        for kc in range(NKC):
                            nc.tensor.matmul(
                                kp_ps[:],
                                wkT[:, kc, ac * P : (ac + 1) * P],
                                kT[:, kc, :],
                                start=(kc == 0),
                                stop=(kc == NKC - 1),
                            )
                        hid = hpool.tile([P, SCW], bf16, tag="hid")
                        nc.scalar.activation(
                            hid[:],
                            kp_ps[:],
                            AF.Tanh,
                            bias=biasT[:, ac, e : e + 1],
                            scale=1.0,
                        )
                        hid_tiles[(sc, ac)] = hid

                # energy for example e: 16 m=1 matmuls col-tiled 4-wide
                # (output row for s-chunk sc at psum partition 32*sc).
                e_ps = en_ps.tile([NPG, SCW], f32, tag="en")
                for ac in range(NAC):
                    for sc in range(NSC):
                        nc.tensor.matmul(
                            e_ps[32 * sc : 32 * sc + 1, :],
                            wa_sb[:, ac : ac + 1],
                            hid_tiles[(sc, ac)][:],
                            start=(ac == 0),
                            stop=(ac == NAC - 1),
                            tile_position=(0, 32 * sc),
                        )
                for sc in range(NSC):
                    nc.vector.tensor_copy(
                        energies[32 * ei : 32 * ei + 1, sc * SCW : (sc + 1) * SCW],
                        e_ps[32 * sc : 32 * sc + 1, :],
                    )

            # ---- group tail: softmax + attn ---------------------------
            nmax = smpool.tile([NPG, 1], f32, tag="nmax")
            nc.vector.reduce_max(nmax[:], energies[:], axis=AX.X, negate=True)
            expv = smpool.tile([NPG, S], f32, tag="expv")
            zsum = smpool.tile([NPG, 1], f32, tag="zsum")
            nc.scalar.activation(
                expv[:],
                energies[:],
                AF.Exp,
                bias=nmax[:, 0:1],
                scale=1.0,
                accum_out=zsum[:, 0:1],
            )
            rz = smpool.tile([NPG, 1], f32, tag="rz")
            nc.vector.reciprocal(rz[:], zsum[:])
            # score_f in true s-order: the on-chip order is s' = (sc, t, p)
            # with s = sc*SCW + 4p + t; permute via the output AP.
            score_f = smpool.tile([NPG, S], f32, tag="score_f")
            nc.vector.tensor_scalar_mul(
                score_f[:].rearrange("n (sc p t) -> n sc t p", p=P, t=4),
                expv[:].rearrange("n (sc t p) -> n sc t p", p=P, t=4),
                rz[:, 0:1],
            )
            score_b = smpool.tile([NPG, S], bf16, tag="score_b")
            nc.vector.tensor_scalar_mul(score_b[:], expv[:], rz[:, 0:1])
            for ei in range(NQ):
                nc.sync.dma_start(
                    score_o[e0 + ei : e0 + ei + 1, :],
                    score_f[32 * ei : 32 * ei + 1, :],
                )

            # scoreT[s, t16, ei] via PE transpose of [NPG, 128] blocks
            scT = smpool.tile([P, S // P, NQ], bf16, tag="scT")
            for t16 in range(S // P):
                ps = tr_ps.tile([P, NPG], bf16, tag="tr")
                nc.tensor.transpose(
                    ps[:],
                    score_b[:, t16 * P : (t16 + 1) * P],
                    ident[:NPG, :NPG],
                )
                nc.vector.tensor_copy(scT[:, t16, :], ps[:, 0 : NPG : 32])

            # attn: 64 m=1 matmuls col-tiled 4-wide (example ei at
            # psum partition 32*ei), then one copy + 4 output DMAs.
            a_ps = en_ps.tile([NPG, D], f32, tag="en")
            for t16 in range(S // P):
                for ei in range(NQ):
                    v_t = v_tiles[(e0 + ei, t16 // NSC)]
                    nc.tensor.matmul(
                        a_ps[32 * ei : 32 * ei + 1, :],
                        scT[:, t16, ei : ei + 1],
                        v_t[:, t16 % NSC, :],
                        start=(t16 == 0),
                        stop=(t16 == S // P - 1),
                        tile_position=(0, 32 * ei),
                    )
            attn_q = smpool.tile([1, NQ * D], f32, tag="attn_q")
            for ei in range(NQ):
                nc.vector.tensor_copy(
                    attn_q[0:1, ei * D : (ei + 1) * D],
                    a_ps[32 * ei : 32 * ei + 1, :],
                )
            nc.sync.dma_start(
                attn_o[e0 : e0 + NQ, 0, :].rearrange("e d -> (e d)")[None, :],
                attn_q[:],
            )

    nc.compile()
    return nc


def _get_nc(reps=1):
    key = ("nc", reps)
    if key not in _CACHE:
        _CACHE[key] = _build(reps)
    return _CACHE[key]


def kernel(k, v, q, Wk, bk, Wq, bq, Wa, ba):
    """Full-input entry point: shards across 8 cores, runs SPMD, gathers."""
    from concourse.bass_utils import run_bass_kernel_spmd

    k = np.asarray(k, dtype=np.float32)
    v = np.asarray(v, dtype=np.float32)
    q = np.asarray(q, dtype=np.float32)
    Wk = np.asarray(Wk, dtype=np.float32)
    bk = np.asarray(bk, dtype=np.float32)
    Wq = np.asarray(Wq, dtype=np.float32)
    bq = np.asarray(bq, dtype=np.float32)
    Wa = np.asarray(Wa, dtype=np.float32)

    nc = _get_nc()
    in_maps = []
    for i in range(NCORES):
        sl = slice(i * E, (i + 1) * E)
        in_maps.append(
            {
                "k": k[sl],
                "v": v[sl],
                "q": q[sl],
                "Wk": Wk,
                "bk": bk,
                "Wq": Wq,
                "bq": bq,
                "Wa": Wa,
            }
        )

    res = run_bass_kernel_spmd(nc, in_maps, core_ids=list(range(NCORES)))
    score = np.concatenate([res.results[i]["score"] for i in range(NCORES)], axis=0)
    attn = np.concatenate([res.results[i]["attn"] for i in range(NCORES)], axis=0)
    return score, attn


# revision 39
# speedup vs baseline: 1.1710x; 1.1710x over previous
"""Additive-attention pooling kernel for Trainium2 (8 NeuronCores, SPMD).

Reference math (per example b):
    kp     = k[b] @ Wk.T + bk          [S, A]
    qp     = q[b] @ Wq.T + bq          [A]
    hidden = tanh(kp + qp)             [S, A]
    energy = hidden @ Wa (+ ba)        [S]     (ba is a constant shift ->
                                                softmax-invariant, dropped)
    score  = softmax(energy)           [S]
    attn   = score @ v[b]              [1, V]

Sharding: data-parallel over batch B=64 -> 8 examples per core, weights
replicated, no cross-core communication.

Per-core dataflow (E=8 examples):
  - k, v streamed from HBM with inline f32->bf16 cast (SWDGE dma).
  - k tiles PE-transposed (contraction dim must sit on partitions).
  - kpT[a, s] = sum_kc WkT[kc,a].T @ kT[kc,s] accumulated in PSUM.
  - hiddenT = tanh(kpT + biasT[:,e]) on ScalarE (per-partition bias holds
    bk + bq + (q @ Wq.T) for this example).
  - energy[1, s] = Wa.T @ hiddenT accumulated over a-chunks.
  - per-pair softmax on the free dim, scores PE-transposed, then
    attn[1, V] = sum_sc scoreT[:,e].T @ v_tile accumulated in PSUM.
"""

import sys

sys.path.insert(0, "/opt/trn_rl_repo")

import numpy as np

# Problem geometry (hardcoded per spec).
B = 64          # global batch
NCORES = 8
E = B // NCORES  # examples per core = 8
S = 2048        # sequence length
D = 512         # k_dim = q_dim = a_dim = v_dim
P = 128         # partitions
NKC = D // P    # contraction chunks = 4
NAC = D // P    # a chunks = 4
SCW = 512       # s-chunk width for the main matmul
NSC = S // SCW  # s-chunks per example = 4
NPAIR = E // 2  # softmax/attn processed per example-pair

_CACHE = {}


def _build(reps=1, layout='pt', prefetch=True, kbufs=4, trb=3, mmb=3, enb=2, vbufs=20, hbufs=20, nq=4, copy_eng='dve'):
    """Build + compile the per-core Bass program (cached per process).

    reps>1 unrolls the whole main pipeline N times inside the NEFF (same
    inputs/outputs) — used by test.py to measure kernel time as the slope
    (T_N - T_1)/(N-1), cancelling the large per-dispatch axon overhead.

    """
    import concourse.bass as bass  # noqa: F401
    import concourse.tile as tile
    from concourse import bacc, mybir
    from concourse.masks import make_identity
    from contextlib import ExitStack

    f32 = mybir.dt.float32
    bf16 = mybir.dt.bfloat16
    AF = mybir.ActivationFunctionType
    AX = mybir.AxisListType

    nc = bacc.Bacc(
        "TRN2",
        target_bir_lowering=False,
        debug=False,
        enable_asserts=True,
        num_devices=NCORES,
    )

    k_d = nc.dram_tensor("k", [E, S, D], f32, kind="ExternalInput").ap()
    v_d = nc.dram_tensor("v", [E, S, D], f32, kind="ExternalInput").ap()
    q_d = nc.dram_tensor("q", [E, D], f32, kind="ExternalInput").ap()
    wk_d = nc.dram_tensor("Wk", [D, D], f32, kind="ExternalInput").ap()
    bk_d = nc.dram_tensor("bk", [D], f32, kind="ExternalInput").ap()
    wq_d = nc.dram_tensor("Wq", [D, D], f32, kind="ExternalInput").ap()
    bq_d = nc.dram_tensor("bq", [D], f32, kind="ExternalInput").ap()
    wa_d = nc.dram_tensor("Wa", [D], f32, kind="ExternalInput").ap()

    score_o = nc.dram_tensor("score", [E, S], f32, kind="ExternalOutput").ap()
    attn_o = nc.dram_tensor("attn", [E, 1, D], f32, kind="ExternalOutput").ap()

    with tile.TileContext(nc) as tc, ExitStack() as ctx:
        const = ctx.enter_context(tc.tile_pool(name="const", bufs=1))

        ident = const.tile([P, P], bf16)
        make_identity(nc, ident[:])
        wkT = const.tile([P, NKC, D], bf16)  # [kk_p, kc, a]
        biasT = const.tile([P, NAC, E], f32)
        wa_sb = const.tile([P, NAC], bf16)

        # ---- main pipeline pools (created first: stack-allocated below
        # the transient setup pools) ------------------------------------
        kpool = ctx.enter_context(tc.tile_pool(name="kpool", bufs=kbufs))
        ktpool = ctx.enter_context(tc.tile_pool(name="ktpool", bufs=kbufs))
        vpool = ctx.enter_context(tc.tile_pool(name="vpool", bufs=vbufs))
        hpool = ctx.enter_context(tc.tile_pool(name="hpool", bufs=hbufs))
        epool = ctx.enter_context(tc.tile_pool(name="epool", bufs=2))
        smpool = ctx.enter_context(tc.tile_pool(name="smpool", bufs=1))

        def load_kv(e, sc):
            """Issue the (cast) k/v chunk loads for (e, sc): one 8KB run
            per partition (s-local = 4*p + t)."""
            spec = "(p t) kk -> p t kk" if layout == "pt" else "(t p) kk -> p t kk"
            k_t = kpool.tile([P, NSC, D], bf16, tag="k")
            nc.gpsimd.dma_start(
                k_t[:],
                k_d[e, sc * SCW : (sc + 1) * SCW, :].rearrange(spec, p=P),
            )
            v_t = vpool.tile([P, NSC, D], bf16, tag="v")
            nc.gpsimd.dma_start(
                v_t[:],
                v_d[e, sc * SCW : (sc + 1) * SCW, :].rearrange(spec, p=P),
            )
            return k_t, v_t

        # prefetch example 0 ahead of the weight preprocessing
        prefetched = {}
        if prefetch:
            for sc in range(NSC):
                prefetched[sc] = load_kv(0, sc)

        # ---- weight preprocessing (scoped pools: freed before main) ----
        with ExitStack() as sctx:
            setup = sctx.enter_context(tc.tile_pool(name="setup", bufs=2))
            ps_setup = sctx.enter_context(
                tc.tile_pool(name="ps_setup", bufs=2, space="PSUM")
            )

            # WkT[kk, a]: PE-transpose of bf16 Wk blocks.
            wk_nat = setup.tile([P, NAC, D], bf16, tag="w_nat")
            nc.gpsimd.dma_start(
                wk_nat[:], wk_d.rearrange("(ab p) kk -> p ab kk", p=P)
            )
            for ab in range(NAC):
                for kc in range(NKC):
                    ps = ps_setup.tile([P, P], bf16, tag="s_ps")
                    nc.tensor.transpose(
                        ps[:], wk_nat[:, ab, kc * P : (kc + 1) * P], ident[:]
                    )
                    nc.vector.tensor_copy(wkT[:, kc, ab * P : (ab + 1) * P], ps[:])

            wq_nat = setup.tile([P, NAC, D], bf16, tag="w_nat")
            nc.gpsimd.dma_start(
                wq_nat[:], wq_d.rearrange("(ab p) qq -> p ab qq", p=P)
            )
            wqT = setup.tile([P, NKC, D], bf16, tag="wqT")  # [qq_p, qc, a]
            for ab in range(NAC):
                for qc in range(NKC):
                    ps = ps_setup.tile([P, P], bf16, tag="s_ps")
                    nc.tensor.transpose(
                        ps[:], wq_nat[:, ab, qc * P : (qc + 1) * P], ident[:]
                    )
                    nc.vector.tensor_copy(wqT[:, qc, ab * P : (ab + 1) * P], ps[:])

            # qT[qq, e]
            q_sb = setup.tile([E, D], bf16, tag="q_sb")
            nc.gpsimd.dma_start(q_sb[:], q_d)
            qT = setup.tile([P, NKC, E], bf16, tag="qT")
            for qc in range(NKC):
                ps = ps_setup.tile([P, E], bf16, tag="s_ps")
                nc.tensor.transpose(
                    ps[:], q_sb[:, qc * P : (qc + 1) * P], ident[:E, :E]
                )
                nc.vector.tensor_copy(qT[:, qc, :], ps[:])

            # biasvec[a] = bk[a] + bq[a], laid out [p, ac] (HWDGE: no cast)
            bk_sb = setup.tile([P, NAC], f32, tag="bk")
            nc.sync.dma_start(bk_sb[:], bk_d.rearrange("(ac p) -> p ac", p=P))
            bq_sb = setup.tile([P, NAC], f32, tag="bq")
            nc.sync.dma_start(bq_sb[:], bq_d.rearrange("(ac p) -> p ac", p=P))
            biasvec = setup.tile([P, NAC], f32, tag="biasvec")
            nc.vector.tensor_add(biasvec[:], bk_sb[:], bq_sb[:])

            # biasT[a, e] = (q @ Wq.T)[e, a] + bk[a] + bq[a]
            for ac in range(NAC):
                pq = ps_setup.tile([P, E], f32, tag="s_ps")
                for qc in range(NKC):
                    nc.tensor.matmul(
                        pq[:],
                        wqT[:, qc, ac * P : (ac + 1) * P],
                        qT[:, qc, :],
                        start=(qc == 0),
                        stop=(qc == NKC - 1),
                    )
                nc.vector.tensor_scalar_add(
                    biasT[:, ac, :], pq[:], biasvec[:, ac : ac + 1]
                )

            # Wa as lhsT chunks [a_p, ac] (bf16 for the energy matmul)
            nc.gpsimd.dma_start(wa_sb[:], wa_d.rearrange("(ac p) -> p ac", p=P))

        # ---- main pipeline --------------------------------------------
        tr_ps = ctx.enter_context(tc.tile_pool(name="tr_ps", bufs=trb, space="PSUM"))
        mm_ps = ctx.enter_context(tc.tile_pool(name="mm_ps", bufs=mmb, space="PSUM"))
        en_ps = ctx.enter_context(tc.tile_pool(name="en_ps", bufs=enb, space="PSUM"))

        v_tiles = {}

        NQ = nq  # examples per tail group
        NPG = 32 * (NQ - 1) + 1  # tail-group tiles: rows at 0/32/...
        NPE = 32 * (NSC - 1) + 1  # energy psum: s-chunk rows at 0/32/64/96
        for rep in range(reps):
          for gr in range(E // NQ):
            e0 = NQ * gr
            energies = epool.tile([NPG, S], f32, tag="energies")
            nc.vector.memset(energies[:], 0.0)

            for ei in range(NQ):
                e = e0 + ei
                hid_tiles = {}
                for sc in range(NSC):
                    if rep == 0 and e == 0 and prefetch:
                        k_t, v_t = prefetched[sc]
                    else:
                        k_t, v_t = load_kv(e, sc)
                    v_tiles[(e, sc)] = v_t

                    # transpose k: kT[kk, kc, s]; 8 blocks (2 t-groups
                    # x 4 kc) fill one PSUM bank -> one copy per 2 t's.
                    kT = ktpool.tile([P, NKC, SCW], bf16, tag="kT")
                    for t0 in range(0, NSC, 2):
                        trp = tr_ps.tile([P, NKC, 2, P], bf16, tag="tr")
                        for t2 in range(2):
                            for kc in range(NKC):
                                nc.tensor.transpose(
                                    trp[:, kc, t2, :],
                                    k_t[:, t0 + t2, kc * P : (kc + 1) * P],
                                    ident[:],
                                )
                        kt_dst = kT[:, :, t0 * P : (t0 + 2) * P].rearrange(
                            "p kc (t s) -> p kc t s", t=2
                        )
                        if copy_eng == "alt" and (sc * 2 + t0 // 2) % 2:
                            nc.scalar.copy(kt_dst, trp[:])
                        else:
                            nc.vector.tensor_copy(kt_dst, trp[:])

                    # kpT[a, s] + tanh (hid kept for the per-example energy)
                    for ac in range(NAC):
                        kp_ps = mm_ps.tile([P, SCW], f32, tag="kp")
                        for kc in range(NKC):
                            nc.tensor.matmul(
                                kp_ps[:],
                                wkT[:, kc, ac * P : (ac + 1) * P],
                                kT[:, kc, :],
                                start=(kc == 0),
                                stop=(kc == NKC - 1),
                            )
                        hid = hpool.tile([P, SCW], bf16, tag="hid")
                        nc.scalar.activation(
                            hid[:],
                            kp_ps[:],
                            AF.Tanh,
                            bias=biasT[:, ac, e : e + 1],
                            scale=1.0,
                        )
                        hid_tiles[(sc, ac)] = hid

                # energy for example e: 16 m=1 matmuls col-tiled 4-wide
                # (output row for s-chunk sc at psum partition 32*sc).
                e_ps = en_ps.tile([NPE, SCW], f32, tag="en")
                for ac in range(NAC):
                    for sc in range(NSC):
                        nc.tensor.matmul(
                            e_ps[32 * sc : 32 * sc + 1, :],
                            wa_sb[:, ac : ac + 1],
                            hid_tiles[(sc, ac)][:],
                            start=(ac == 0),
                            stop=(ac == NAC - 1),
                            tile_position=(0, 32 * sc),
                        )
                for sc in range(NSC):
                    nc.vector.tensor_copy(
                        energies[32 * ei : 32 * ei + 1, sc * SCW : (sc + 1) * SCW],
                        e_ps[32 * sc : 32 * sc + 1, :],
                    )

            # ---- group tail: softmax + attn ---------------------------
            nmax = smpool.tile([NPG, 1], f32, tag="nmax")
            nc.vector.reduce_max(nmax[:], energies[:], axis=AX.X, negate=True)
            expv = smpool.tile([NPG, S], f32, tag="expv")
            zsum = smpool.tile([NPG, 1], f32, tag="zsum")
            nc.scalar.activation(
                expv[:],
                energies[:],
                AF.Exp,
                bias=nmax[:, 0:1],
                scale=1.0,
                accum_out=zsum[:, 0:1],
            )
            rz = smpool.tile([NPG, 1], f32, tag="rz")
            nc.vector.reciprocal(rz[:], zsum[:])
            # score_f in true s-order: the on-chip order is s' = (sc, t, p)
            # with s = sc*SCW + 4p + t; permute via the output AP.
            score_f = smpool.tile([NPG, S], f32, tag="score_f")
            if layout == "pt":
                nc.vector.tensor_scalar_mul(
                    score_f[:].rearrange("n (sc p t) -> n sc t p", p=P, t=4),
                    expv[:].rearrange("n (sc t p) -> n sc t p", p=P, t=4),
                    rz[:, 0:1],
                )
            else:
                nc.vector.tensor_scalar_mul(score_f[:], expv[:], rz[:, 0:1])
            score_b = smpool.tile([NPG, S], bf16, tag="score_b")
            nc.vector.tensor_scalar_mul(score_b[:], expv[:], rz[:, 0:1])
            for ei in range(NQ):
                nc.sync.dma_start(
                    score_o[e0 + ei : e0 + ei + 1, :],
                    score_f[32 * ei : 32 * ei + 1, :],
                )

            # scoreT[s, t16, ei] via PE transpose of [NPG, 128] blocks
            scT = smpool.tile([P, S // P, NQ], bf16, tag="scT")
            for t16 in range(S // P):
                ps = tr_ps.tile([P, NPG], bf16, tag="tr")
                nc.tensor.transpose(
                    ps[:],
                    score_b[:, t16 * P : (t16 + 1) * P],
                    ident[:NPG, :NPG],
                )
                nc.vector.tensor_copy(scT[:, t16, :], ps[:, 0 : NPG : 32])

            # attn: 64 m=1 matmuls col-tiled 4-wide (example ei at
            # psum partition 32*ei), then one copy + 4 output DMAs.
            a_ps = en_ps.tile([NPG, D], f32, tag="en")
            for t16 in range(S // P):
                for ei in range(NQ):
                    v_t = v_tiles[(e0 + ei, t16 // NSC)]
                    nc.tensor.matmul(
                        a_ps[32 * ei : 32 * ei + 1, :],
                        scT[:, t16, ei : ei + 1],
                        v_t[:, t16 % NSC, :],
                        start=(t16 == 0),
                        stop=(t16 == S // P - 1),
                        tile_position=(0, 32 * ei),
                    )
            attn_q = smpool.tile([1, NQ * D], f32, tag="attn_q")
            for ei in range(NQ):
                nc.vector.tensor_copy(
                    attn_q[0:1, ei * D : (ei + 1) * D],
                    a_ps[32 * ei : 32 * ei + 1, :],
                )
            nc.sync.dma_start(
                attn_o[e0 : e0 + NQ, 0, :].rearrange("e d -> (e d)")[None, :],
                attn_q[:],
            )

    nc.compile()
    return nc


def _get_nc(reps=1, **kw):
    key = ("nc", reps, tuple(sorted(kw.items())))
    if key not in _CACHE:
        _CACHE[key] = _build(reps, **kw)
    return _CACHE[key]


def kernel(k, v, q, Wk, bk, Wq, bq, Wa, ba):
    """Full-input entry point: shards across 8 cores, runs SPMD, gathers."""
    from concourse.bass_utils import run_bass_kernel_spmd

    k = np.asarray(k, dtype=np.float32)
    v = np.asarray(v, dtype=np.float32)
    q = np.asarray(q, dtype=np.float32)
    Wk = np.asarray(Wk, dtype=np.float32)
    bk = np.asarray(bk, dtype=np.float32)
    Wq = np.asarray(Wq, dtype=np.float32)
    bq = np.asarray(bq, dtype=np.float32)
    Wa = np.asarray(Wa, dtype=np.float32)

    nc = _get_nc()
    in_maps = []
    for i in range(NCORES):
        sl = slice(i * E, (i + 1) * E)
        in_maps.append(
            {
                "k": k[sl],
                "v": v[sl],
                "q": q[sl],
                "Wk": Wk,
                "bk": bk,
                "Wq": Wq,
                "bq": bq,
                "Wa": Wa,
            }
        )

    last_err = None
    for _attempt in range(3):
        try:
            res = run_bass_kernel_spmd(nc, in_maps, core_ids=list(range(NCORES)))
            break
        except Exception as ex:  # transient device/tunnel failures
            last_err = ex
    else:
        raise last_err
    score = np.concatenate([res.results[i]["score"] for i in range(NCORES)], axis=0)
    attn = np.concatenate([res.results[i]["attn"] for i in range(NCORES)], axis=0)
    return score, attn


# revision 40
# speedup vs baseline: 2.0140x; 1.7199x over previous
"""Additive-attention pooling kernel for Trainium2 (8 NeuronCores, SPMD).

Reference math (per example b):
    kp     = k[b] @ Wk.T + bk          [S, A]
    qp     = q[b] @ Wq.T + bq          [A]
    hidden = tanh(kp + qp)             [S, A]
    energy = hidden @ Wa (+ ba)        [S]     (ba is a constant shift ->
                                                softmax-invariant, dropped)
    score  = softmax(energy)           [S]
    attn   = score @ v[b]              [1, V]

Sharding: data-parallel over batch B=64 -> 8 examples per core, weights
replicated, no cross-core communication.

Per-core dataflow (E=8 examples):
  - k, v streamed from HBM with inline f32->bf16 cast (SWDGE dma).
  - k tiles PE-transposed (contraction dim must sit on partitions).
  - kpT[a, s] = sum_kc WkT[kc,a].T @ kT[kc,s] accumulated in PSUM.
  - hiddenT = tanh(kpT + biasT[:,e]) on ScalarE (per-partition bias holds
    bk + bq + (q @ Wq.T) for this example).
  - energy[1, s] = Wa.T @ hiddenT accumulated over a-chunks.
  - per-quad softmax on the free dim (example rows at partitions
    0/32/64/96), scores PE-transposed, then attn[1, V] = sum_sc
    scoreT[:,e].T @ v_tile accumulated in PSUM (col-tiled 4-wide).
"""

import sys

sys.path.insert(0, "/opt/trn_rl_repo")

import numpy as np

# Problem geometry (hardcoded per spec).
B = 64          # global batch
NCORES = 8
E = B // NCORES  # examples per core = 8
S = 2048        # sequence length
D = 512         # k_dim = q_dim = a_dim = v_dim
P = 128         # partitions
NKC = D // P    # contraction chunks = 4
NAC = D // P    # a chunks = 4
SCW = 512       # s-chunk width for the main matmul
NSC = S // SCW  # s-chunks per example = 4
NPAIR = E // 2  # softmax/attn processed per example-pair

_CACHE = {}


def _build(reps=1, layout='pt', prefetch=False, kbufs=4, trb=3, mmb=3, enb=2, vbufs=20, hbufs=20, nq=4, copy_eng='dve'):
    """Build + compile the per-core Bass program (cached per process).

    reps>1 unrolls the whole main pipeline N times inside the NEFF (same
    inputs/outputs) — used by test.py to measure kernel time as the slope
    (T_N - T_1)/(N-1), cancelling the large per-dispatch axon overhead.

    """
    import concourse.bass as bass  # noqa: F401
    import concourse.tile as tile
    from concourse import bacc, mybir
    from concourse.masks import make_identity
    from contextlib import ExitStack

    f32 = mybir.dt.float32
    bf16 = mybir.dt.bfloat16
    AF = mybir.ActivationFunctionType
    AX = mybir.AxisListType

    nc = bacc.Bacc(
        "TRN2",
        target_bir_lowering=False,
        debug=False,
        enable_asserts=True,
        num_devices=NCORES,
    )

    k_d = nc.dram_tensor("k", [E, S, D], f32, kind="ExternalInput").ap()
    v_d = nc.dram_tensor("v", [E, S, D], f32, kind="ExternalInput").ap()
    q_d = nc.dram_tensor("q", [E, D], f32, kind="ExternalInput").ap()
    wk_d = nc.dram_tensor("Wk", [D, D], f32, kind="ExternalInput").ap()
    bk_d = nc.dram_tensor("bk", [D], f32, kind="ExternalInput").ap()
    wq_d = nc.dram_tensor("Wq", [D, D], f32, kind="ExternalInput").ap()
    bq_d = nc.dram_tensor("bq", [D], f32, kind="ExternalInput").ap()
    wa_d = nc.dram_tensor("Wa", [D], f32, kind="ExternalInput").ap()

    score_o = nc.dram_tensor("score", [E, S], f32, kind="ExternalOutput").ap()
    attn_o = nc.dram_tensor("attn", [E, 1, D], f32, kind="ExternalOutput").ap()

    with tile.TileContext(nc) as tc, ExitStack() as ctx:
        const = ctx.enter_context(tc.tile_pool(name="const", bufs=1))

        ident = const.tile([P, P], bf16)
        make_identity(nc, ident[:])
        wkT = const.tile([P, NKC, D], bf16)  # [kk_p, kc, a]
        biasT = const.tile([P, NAC, E], f32)
        wa_sb = const.tile([P, NAC], bf16)

        # ---- main pipeline pools (created first: stack-allocated below
        # the transient setup pools) ------------------------------------
        kpool = ctx.enter_context(tc.tile_pool(name="kpool", bufs=kbufs))
        ktpool = ctx.enter_context(tc.tile_pool(name="ktpool", bufs=kbufs))
        vpool = ctx.enter_context(tc.tile_pool(name="vpool", bufs=vbufs))
        hpool = ctx.enter_context(tc.tile_pool(name="hpool", bufs=hbufs))
        epool = ctx.enter_context(tc.tile_pool(name="epool", bufs=2))
        smpool = ctx.enter_context(tc.tile_pool(name="smpool", bufs=1))

        def load_kv(e, sc):
            """Issue the (cast) k/v chunk loads for (e, sc): one 8KB run
            per partition (s-local = 4*p + t)."""
            spec = "(p t) kk -> p t kk" if layout == "pt" else "(t p) kk -> p t kk"
            k_t = kpool.tile([P, NSC, D], bf16, tag="k")
            nc.gpsimd.dma_start(
                k_t[:],
                k_d[e, sc * SCW : (sc + 1) * SCW, :].rearrange(spec, p=P),
            )
            v_t = vpool.tile([P, NSC, D], bf16, tag="v")
            nc.gpsimd.dma_start(
                v_t[:],
                v_d[e, sc * SCW : (sc + 1) * SCW, :].rearrange(spec, p=P),
            )
            return k_t, v_t

        # prefetch example 0 ahead of the weight preprocessing
        prefetched = {}
        if prefetch:
            for sc in range(NSC):
                prefetched[sc] = load_kv(0, sc)

        # ---- weight preprocessing (scoped pools: freed before main) ----
        with ExitStack() as sctx:
            setup = sctx.enter_context(tc.tile_pool(name="setup", bufs=2))
            ps_setup = sctx.enter_context(
                tc.tile_pool(name="ps_setup", bufs=2, space="PSUM")
            )

            # WkT[kk, a]: PE-transpose of bf16 Wk blocks.
            wk_nat = setup.tile([P, NAC, D], bf16, tag="w_nat")
            nc.gpsimd.dma_start(
                wk_nat[:], wk_d.rearrange("(ab p) kk -> p ab kk", p=P)
            )
            for ab in range(NAC):
                for kc in range(NKC):
                    ps = ps_setup.tile([P, P], bf16, tag="s_ps")
                    nc.tensor.transpose(
                        ps[:], wk_nat[:, ab, kc * P : (kc + 1) * P], ident[:]
                    )
                    nc.vector.tensor_copy(wkT[:, kc, ab * P : (ab + 1) * P], ps[:])

            wq_nat = setup.tile([P, NAC, D], bf16, tag="w_nat")
            nc.gpsimd.dma_start(
                wq_nat[:], wq_d.rearrange("(ab p) qq -> p ab qq", p=P)
            )
            wqT = setup.tile([P, NKC, D], bf16, tag="wqT")  # [qq_p, qc, a]
            for ab in range(NAC):
                for qc in range(NKC):
                    ps = ps_setup.tile([P, P], bf16, tag="s_ps")
                    nc.tensor.transpose(
                        ps[:], wq_nat[:, ab, qc * P : (qc + 1) * P], ident[:]
                    )
                    nc.vector.tensor_copy(wqT[:, qc, ab * P : (ab + 1) * P], ps[:])

            # qT[qq, e]
            q_sb = setup.tile([E, D], bf16, tag="q_sb")
            nc.gpsimd.dma_start(q_sb[:], q_d)
            qT = setup.tile([P, NKC, E], bf16, tag="qT")
            for qc in range(NKC):
                ps = ps_setup.tile([P, E], bf16, tag="s_ps")
                nc.tensor.transpose(
                    ps[:], q_sb[:, qc * P : (qc + 1) * P], ident[:E, :E]
                )
                nc.vector.tensor_copy(qT[:, qc, :], ps[:])

            # biasvec[a] = bk[a] + bq[a], laid out [p, ac] (HWDGE: no cast)
            bk_sb = setup.tile([P, NAC], f32, tag="bk")
            nc.sync.dma_start(bk_sb[:], bk_d.rearrange("(ac p) -> p ac", p=P))
            bq_sb = setup.tile([P, NAC], f32, tag="bq")
            nc.sync.dma_start(bq_sb[:], bq_d.rearrange("(ac p) -> p ac", p=P))
            biasvec = setup.tile([P, NAC], f32, tag="biasvec")
            nc.vector.tensor_add(biasvec[:], bk_sb[:], bq_sb[:])

            # biasT[a, e] = (q @ Wq.T)[e, a] + bk[a] + bq[a]
            for ac in range(NAC):
                pq = ps_setup.tile([P, E], f32, tag="s_ps")
                for qc in range(NKC):
                    nc.tensor.matmul(
                        pq[:],
                        wqT[:, qc, ac * P : (ac + 1) * P],
                        qT[:, qc, :],
                        start=(qc == 0),
                        stop=(qc == NKC - 1),
                    )
                nc.vector.tensor_scalar_add(
                    biasT[:, ac, :], pq[:], biasvec[:, ac : ac + 1]
                )

            # Wa as lhsT chunks [a_p, ac] (bf16 for the energy matmul)
            nc.gpsimd.dma_start(wa_sb[:], wa_d.rearrange("(ac p) -> p ac", p=P))

        # ---- main pipeline --------------------------------------------
        tr_ps = ctx.enter_context(tc.tile_pool(name="tr_ps", bufs=trb, space="PSUM"))
        mm_ps = ctx.enter_context(tc.tile_pool(name="mm_ps", bufs=mmb, space="PSUM"))
        en_ps = ctx.enter_context(tc.tile_pool(name="en_ps", bufs=enb, space="PSUM"))

        v_tiles = {}

        NQ = nq  # examples per tail group
        NPG = 32 * (NQ - 1) + 1  # tail-group tiles: rows at 0/32/...
        NPE = 32 * (NSC - 1) + 1  # energy psum: s-chunk rows at 0/32/64/96
        for rep in range(reps):
          for gr in range(E // NQ):
            e0 = NQ * gr
            energies = epool.tile([NPG, S], f32, tag="energies")
            nc.vector.memset(energies[:], 0.0)

            for ei in range(NQ):
                e = e0 + ei
                hid_tiles = {}
                for sc in range(NSC):
                    if rep == 0 and e == 0 and prefetch:
                        k_t, v_t = prefetched[sc]
                    else:
                        k_t, v_t = load_kv(e, sc)
                    v_tiles[(e, sc)] = v_t

                    # transpose k: kT[kk, kc, s]; 8 blocks (2 t-groups
                    # x 4 kc) fill one PSUM bank -> one copy per 2 t's.
                    kT = ktpool.tile([P, NKC, SCW], bf16, tag="kT")
                    for t0 in range(0, NSC, 2):
                        trp = tr_ps.tile([P, NKC, 2, P], bf16, tag="tr")
                        for t2 in range(2):
                            for kc in range(NKC):
                                nc.tensor.transpose(
                                    trp[:, kc, t2, :],
                                    k_t[:, t0 + t2, kc * P : (kc + 1) * P],
                                    ident[:],
                                )
                        kt_dst = kT[:, :, t0 * P : (t0 + 2) * P].rearrange(
                            "p kc (t s) -> p kc t s", t=2
                        )
                        if copy_eng == "alt" and (sc * 2 + t0 // 2) % 2:
                            nc.scalar.copy(kt_dst, trp[:])
                        else:
                            nc.vector.tensor_copy(kt_dst, trp[:])

                    # kpT[a, s] + tanh (hid kept for the per-example energy)
                    for ac in range(NAC):
                        kp_ps = mm_ps.tile([P, SCW], f32, tag="kp")
                        for kc in range(NKC):
                            nc.tensor.matmul(
                                kp_ps[:],
                                wkT[:, kc, ac * P : (ac + 1) * P],
                                kT[:, kc, :],
                                start=(kc == 0),
                                stop=(kc == NKC - 1),
                            )
                        hid = hpool.tile([P, SCW], bf16, tag="hid")
                        nc.scalar.activation(
                            hid[:],
                            kp_ps[:],
                            AF.Tanh,
                            bias=biasT[:, ac, e : e + 1],
                            scale=1.0,
                        )
                        hid_tiles[(sc, ac)] = hid

                # energy for example e: 16 m=1 matmuls col-tiled 4-wide
                # (output row for s-chunk sc at psum partition 32*sc).
                e_ps = en_ps.tile([NPE, SCW], f32, tag="en")
                for ac in range(NAC):
                    for sc in range(NSC):
                        nc.tensor.matmul(
                            e_ps[32 * sc : 32 * sc + 1, :],
                            wa_sb[:, ac : ac + 1],
                            hid_tiles[(sc, ac)][:],
                            start=(ac == 0),
                            stop=(ac == NAC - 1),
                            tile_position=(0, 32 * sc),
                        )
                for sc in range(NSC):
                    nc.vector.tensor_copy(
                        energies[32 * ei : 32 * ei + 1, sc * SCW : (sc + 1) * SCW],
                        e_ps[32 * sc : 32 * sc + 1, :],
                    )

            # ---- group tail: softmax + attn ---------------------------
            nmax = smpool.tile([NPG, 1], f32, tag="nmax")
            nc.vector.reduce_max(nmax[:], energies[:], axis=AX.X, negate=True)
            expv = smpool.tile([NPG, S], f32, tag="expv")
            zsum = smpool.tile([NPG, 1], f32, tag="zsum")
            nc.scalar.activation(
                expv[:],
                energies[:],
                AF.Exp,
                bias=nmax[:, 0:1],
                scale=1.0,
                accum_out=zsum[:, 0:1],
            )
            rz = smpool.tile([NPG, 1], f32, tag="rz")
            nc.vector.reciprocal(rz[:], zsum[:])
            # score_f in true s-order: the on-chip order is s' = (sc, t, p)
            # with s = sc*SCW + 4p + t; permute via the output AP.
            score_f = smpool.tile([NPG, S], f32, tag="score_f")
            if layout == "pt":
                nc.vector.tensor_scalar_mul(
                    score_f[:].rearrange("n (sc p t) -> n sc t p", p=P, t=4),
                    expv[:].rearrange("n (sc t p) -> n sc t p", p=P, t=4),
                    rz[:, 0:1],
                )
            else:
                nc.vector.tensor_scalar_mul(score_f[:], expv[:], rz[:, 0:1])
            score_b = smpool.tile([NPG, S], bf16, tag="score_b")
            nc.vector.tensor_scalar_mul(score_b[:], expv[:], rz[:, 0:1])
            for ei in range(NQ):
                nc.sync.dma_start(
                    score_o[e0 + ei : e0 + ei + 1, :],
                    score_f[32 * ei : 32 * ei + 1, :],
                )

            # scoreT[s, t16, ei] via PE transpose of [NPG, 128] blocks
            scT = smpool.tile([P, S // P, NQ], bf16, tag="scT")
            for t16 in range(S // P):
                ps = tr_ps.tile([P, NPG], bf16, tag="tr")
                nc.tensor.transpose(
                    ps[:],
                    score_b[:, t16 * P : (t16 + 1) * P],
                    ident[:NPG, :NPG],
                )
                nc.vector.tensor_copy(scT[:, t16, :], ps[:, 0 : NPG : 32])

            # attn: 64 m=1 matmuls col-tiled 4-wide (example ei at
            # psum partition 32*ei), then one copy + 4 output DMAs.
            a_ps = en_ps.tile([NPG, D], f32, tag="en")
            for t16 in range(S // P):
                for ei in range(NQ):
                    v_t = v_tiles[(e0 + ei, t16 // NSC)]
                    nc.tensor.matmul(
                        a_ps[32 * ei : 32 * ei + 1, :],
                        scT[:, t16, ei : ei + 1],
                        v_t[:, t16 % NSC, :],
                        start=(t16 == 0),
                        stop=(t16 == S // P - 1),
                        tile_position=(0, 32 * ei),
                    )
            attn_q = smpool.tile([1, NQ * D], f32, tag="attn_q")
            for ei in range(NQ):
                nc.vector.tensor_copy(
                    attn_q[0:1, ei * D : (ei + 1) * D],
                    a_ps[32 * ei : 32 * ei + 1, :],
                )
            nc.sync.dma_start(
                attn_o[e0 : e0 + NQ, 0, :].rearrange("e d -> (e d)")[None, :],
                attn_q[:],
            )

    nc.compile()
    return nc


def _get_nc(reps=1, **kw):
    key = ("nc", reps, tuple(sorted(kw.items())))
    if key not in _CACHE:
        _CACHE[key] = _build(reps, **kw)
    return _CACHE[key]


def kernel(k, v, q, Wk, bk, Wq, bq, Wa, ba):
    """Full-input entry point: shards across 8 cores, runs SPMD, gathers."""
    from concourse.bass_utils import run_bass_kernel_spmd

    k = np.asarray(k, dtype=np.float32)
    v = np.asarray(v, dtype=np.float32)
    q = np.asarray(q, dtype=np.float32)
    Wk = np.asarray(Wk, dtype=np.float32)
    bk = np.asarray(bk, dtype=np.float32)
    Wq = np.asarray(Wq, dtype=np.float32)
    bq = np.asarray(bq, dtype=np.float32)
    Wa = np.asarray(Wa, dtype=np.float32)

    nc = _get_nc()
    in_maps = []
    for i in range(NCORES):
        sl = slice(i * E, (i + 1) * E)
        in_maps.append(
            {
                "k": k[sl],
                "v": v[sl],
                "q": q[sl],
                "Wk": Wk,
                "bk": bk,
                "Wq": Wq,
                "bq": bq,
                "Wa": Wa,
            }
        )

    last_err = None
    for _attempt in range(3):
        try:
            res = run_bass_kernel_spmd(nc, in_maps, core_ids=list(range(NCORES)))
            break
        except Exception as ex:  # transient device/tunnel failures
            last_err = ex
    else:
        raise last_err
    score = np.concatenate([res.results[i]["score"] for i in range(NCORES)], axis=0)
    attn = np.concatenate([res.results[i]["attn"] for i in range(NCORES)], axis=0)
    return score, attn
